# revision 8
# baseline (speedup 1.0000x reference)
"""Multi-hot embedding bag kernel for Trainium2 (8 NeuronCores, batch-sharded).

Computes, for 5 feature groups g with multi-hot int32 matrices A_g [B, V_g]
and weights W_g [V_g, 64]:
    out = concat_g(norm_g(A_g @ W_g))  with the original module's quirks:
    - "decades" is normalized by its own row-sum AND by the movie row-sum
    - "movies" is never normalized
    - remaining groups are normalized by their own row-sum (rows with sum 0
      are left unnormalized)

Strategy per core (256 batch rows):
  - A is transposed on the HOST (int32 preserved) into [128, C, 256]: vocab
    on partitions, one 128-vocab chunk per 256-batch column block, all 5
    groups concatenated chunk-wise with zero padding. The kernel streams
    group-aligned slabs of chunks HBM->SBUF via gpsimd (SWDGE) DMA with an
    int32 -> fp16 cast ({0,1} values are exact; the int32->fp8 cast DMA
    measured ~20% slower per source byte, so fp16 it is).
  - W is packed host-side per chunk as [W | 1] (the ones column accumulates
    the multi-hot row-sums): fp16 for movies (unnormalized output, sets the
    global scale), fp8e4m3 for the 4 normalized groups (quantization error
    is divided by ~row-sum, negligible; the PE takes mixed fp8 stationary x
    fp16 moving operands). Halves most of the W DMA traffic.
  - per chunk, ONE matmul accumulates accT [65, 256] in PSUM (lhsT = W-chunk
    stationary, rhs = A^T-chunk moving); a group's chunks form one PSUM
    accumulation group.
  - at group end accT is copied to SBUF, transposed back on the PE (fp32
    identity) to [256, 65], then normalized with per-row reciprocals.
  The 16 DMA queues are the only saturated resource (the int32 A read is
  the roofline); "cat" (1 chunk) goes last so the tail epilogue is short.
"""

import math

import numpy as np

import concourse.bass as bass
import concourse.tile as tile
from concourse import bacc, mybir
from concourse.bass_utils import run_bass_kernel_spmd
from concourse.masks import make_identity

B = 2048
LF = 64
FE = LF + 1  # weights + ones column
N_CORES = 8
BPC = B // N_CORES  # 256 batch rows per core
P = 128
SLAB = 32  # vocab chunks per A-slab DMA (32 -> 4 MiB int32 reads)

_FP16 = mybir.dt.float16
_FP32 = mybir.dt.float32
_FP8 = mybir.dt.float8e4

# (key, idx input name, weight input name, vocab size, output column offset,
#  on-chip dtype). Movies first so its row-sum reciprocal exists when decades
# is normalized; categories (1 chunk) last so the final epilogue tail is
# short.
GROUPS = [
    ("mov", "movie_idxs", "W_mov", 60000, 64, _FP16),
    ("dec", "decade_idxs", "W_dec", 12, 0, _FP8),
    ("per", "person_idxs", "W_per", 100000, 192, _FP8),
    ("com", "company_idxs", "W_com", 20000, 256, _FP8),
    ("cat", "category_idxs", "W_cat", 32, 128, _FP8),
]
N_CH = [math.ceil(v / P) for _, _, _, v, _, _ in GROUPS]
C_TOT = sum(N_CH)  # 1410 chunks of 128 vocab rows
C_16 = N_CH[0]  # fp16 (movie) chunks
C_8 = C_TOT - C_16  # fp8 chunks
OUT_COLS = 5 * LF


def _build() -> bass.Bass:
    nc = bacc.Bacc(None, target_bir_lowering=False)

    a_dram = nc.dram_tensor("a_all", [P, C_TOT * 2 * P], mybir.dt.int32,
                            kind="ExternalInput")
    w16_dram = nc.dram_tensor("w16", [P, C_16 * FE], _FP16,
                              kind="ExternalInput")
    w8_dram = nc.dram_tensor("w8", [P, C_8 * FE], _FP8, kind="ExternalInput")
    out = nc.dram_tensor("out", [BPC, OUT_COLS], _FP32, kind="ExternalOutput")

    with tile.TileContext(nc) as tc:
        with (
            tc.tile_pool(name="singles", bufs=1) as singles,
            tc.tile_pool(name="apool", bufs=4) as apool,
            tc.tile_pool(name="wpool", bufs=4) as wpool,
            tc.tile_pool(name="npool", bufs=4) as npool,
            tc.tile_pool(name="accp", bufs=3, space="PSUM") as accp,
            tc.tile_pool(name="backp", bufs=2, space="PSUM") as backp,
        ):
            ident32 = singles.tile([P, P], _FP32)
            out_sb = [singles.tile([P, OUT_COLS], _FP32, name=f"out_sb{i}")
                      for i in range(2)]
            rmov = [singles.tile([P, 1], _FP32, name=f"rmov{i}")
                    for i in range(2)]

            c0 = 0  # global chunk cursor
            w8_c0 = 0  # chunk cursor within the fp8 weight tensor
            first = True
            for gi, (key, _, _, v, col, gdt) in enumerate(GROUPS):
                nch = N_CH[gi]
                accT = accp.tile([FE, 2 * P], _FP32, tag="acc",
                                 name=f"accT_{key}")
                for s0 in range(0, nch, SLAB):
                    ch = min(SLAB, nch - s0)
                    a_sb = apool.tile([P, SLAB, 2 * P], _FP16, tag="a")
                    nc.gpsimd.dma_start(
                        a_sb[:, :ch, :],
                        a_dram[:, (c0 + s0) * 2 * P:(c0 + s0 + ch) * 2 * P]
                        .rearrange("p (c b) -> p c b", b=2 * P),
                    )
                    w_sb = wpool.tile([P, SLAB, FE], gdt, tag=f"w{gdt.name}")
                    wdram = w16_dram if gdt == _FP16 else w8_dram
                    wb = s0 if gdt == _FP16 else w8_c0 + s0
                    nc.sync.dma_start(
                        w_sb[:, :ch, :],
                        wdram[:, wb * FE:(wb + ch) * FE].rearrange(
                            "p (c f) -> p c f", f=FE),
                    )
                    if first:
                        # after the first slab DMAs so identity construction
                        # (gpsimd) doesn't delay the first SWDGE descriptors
                        make_identity(nc, ident32)
                        first = False
                    for j in range(ch):
                        cidx = s0 + j
                        nc.tensor.matmul(
                            accT,
                            lhsT=w_sb[:, j, :],
                            rhs=a_sb[:, j, :],
                            start=(cidx == 0),
                            stop=(cidx == nch - 1),
                        )
                c0 += nch
                if gdt != _FP16:
                    w8_c0 += nch

                # group epilogue: back-transpose, normalize, stage output
                accT_sb = npool.tile([FE, 2 * P], _FP32, tag="accsb")
                nc.vector.tensor_copy(accT_sb, accT)
                for bt in range(2):
                    out2 = backp.tile([P, FE], _FP32, tag="out2")
                    nc.tensor.matmul(
                        out2,
                        lhsT=accT_sb[:, bass.ts(bt, P)],
                        rhs=ident32[:FE, :FE],
                        start=True, stop=True,
                    )
                    s = npool.tile([P, 1], _FP32, tag="s")
                    nc.vector.tensor_scalar_max(s, out2[:, LF:FE], 1.0)
                    nc.vector.reciprocal(s, s)
                    if key == "mov":
                        # movies are left unnormalized; stash 1/max(sum,1)
                        # for the decades double-normalization
                        nc.vector.tensor_copy(rmov[bt], s)
                        nc.scalar.copy(out_sb[bt][:, col:col + LF],
                                       out2[:, :LF])
                    else:
                        if key == "dec":
                            nc.vector.tensor_mul(s, s, rmov[bt])
                        nc.vector.tensor_scalar_mul(
                            out_sb[bt][:, col:col + LF], out2[:, :LF], s)

            for bt in range(2):
                nc.sync.dma_start(out[bt * P:(bt + 1) * P, :], out_sb[bt])

    nc.finalize()
    return nc


_NC_CACHE: bass.Bass | None = None


def _get_nc() -> bass.Bass:
    global _NC_CACHE
    if _NC_CACHE is None:
        _NC_CACHE = _build()
    return _NC_CACHE


def _pack_weights(inputs: dict) -> dict[str, np.ndarray]:
    """Chunk-major [W_g | 1] packs: fp16 for movies, fp8e4m3 for the rest."""
    w16 = np.zeros((P, C_16, FE), np.float16)
    w8 = np.zeros((P, C_8, FE), mybir.dt.np(_FP8))
    c16 = c8 = 0
    for (_, _, wname, v, _, gdt), c in zip(GROUPS, N_CH):
        we = np.concatenate(
            [np.asarray(inputs[wname], np.float32),
             np.ones((v, 1), np.float32)], axis=1)
        if c * P > v:
            we = np.concatenate(
                [we, np.zeros((c * P - v, FE), np.float32)], axis=0)
        chunked = we.reshape(c, P, FE).transpose(1, 0, 2)
        if gdt == _FP16:
            w16[:, c16:c16 + c, :] = chunked.astype(np.float16)
            c16 += c
        else:
            w8[:, c8:c8 + c, :] = chunked.astype(mybir.dt.np(_FP8))
            c8 += c
    return {
        "w16": np.ascontiguousarray(w16.reshape(P, C_16 * FE)),
        "w8": np.ascontiguousarray(w8.reshape(P, C_8 * FE)),
    }


def _pack_a(inputs: dict) -> np.ndarray:
    """Host transpose (int32 preserved): per core, vocab chunks on partitions.

    Returns [N_CORES, 128, C_TOT, 2*P] int32 where [core, p, c0g+c, b] =
    A_g[core*256 + b, c*128 + p] (zero beyond each group's vocab)."""
    arr = np.zeros((N_CORES, P, C_TOT, 2 * P), np.int32)
    c0 = 0
    for (_, aname, _, v, _, _), c in zip(GROUPS, N_CH):
        a = np.asarray(inputs[aname], np.int32).reshape(N_CORES, 2 * P, v)
        fc = v // P
        if fc:
            arr[:, :, c0:c0 + fc, :] = (
                a[:, :, :fc * P].reshape(N_CORES, 2 * P, fc, P)
                .transpose(0, 3, 2, 1))
        if v % P:
            arr[:, :v % P, c0 + fc, :] = a[:, :, fc * P:].transpose(0, 2, 1)
        c0 += c
    return arr


def kernel(**inputs: np.ndarray) -> np.ndarray:
    import os

    nc = _get_nc()

    w_packs = _pack_weights(inputs)
    a_all = _pack_a(inputs)

    in_maps = []
    for core in range(N_CORES):
        m = dict(w_packs)
        m["a_all"] = a_all[core].reshape(P, C_TOT * 2 * P)
        in_maps.append(m)

    trace = bool(int(os.environ.get("EMB_TRACE", "0")))
    res = run_bass_kernel_spmd(nc, in_maps, core_ids=list(range(N_CORES)),
                               trace=trace)
    if trace and res.exec_time_ns is not None:
        print(f"HW exec time: {res.exec_time_ns} ns")
        if res.instructions_and_trace is not None:
            print(f"trace: {res.instructions_and_trace[1]}")

    return np.concatenate([r["out"] for r in res.results], axis=0)


# revision 9
# speedup vs baseline: 1.0680x; 1.0680x over previous
"""Multi-hot embedding bag kernel for Trainium2 (8 NeuronCores, batch-sharded).

Computes, for 5 feature groups g with multi-hot int32 matrices A_g [B, V_g]
and weights W_g [V_g, 64]:
    out = concat_g(norm_g(A_g @ W_g))  with the original module's quirks:
    - "decades" is normalized by its own row-sum AND by the movie row-sum
    - "movies" is never normalized
    - remaining groups are normalized by their own row-sum (rows with sum 0
      are left unnormalized)

Strategy per core (256 batch rows):
  - A is transposed on the HOST (int32 preserved) into [128, C, 256]: vocab
    on partitions, one 128-vocab chunk per 256-batch column block, all 5
    groups concatenated chunk-wise with zero padding. The kernel streams
    slabs of chunks HBM->SBUF via gpsimd (SWDGE) DMA with int32->fp16 cast.
  - W is packed host-side as [128, C*65] fp16 ([W | 1] per chunk; the ones
    column accumulates the multi-hot row-sums).
  - per chunk, ONE fp16 matmul accumulates accT [65, 256] in PSUM
    (lhsT = W-chunk stationary, rhs = A^T-chunk moving); vocab chunks of a
    group form one PSUM accumulation group.
  - at group end accT is copied to SBUF, transposed back on the PE (fp32
    identity) to [256, 65], then normalized with per-row reciprocals.
  This leaves the 16 DMA queues as the only saturated resource (the int32
  A read is the roofline).
"""

import math

import numpy as np

import concourse.bass as bass
import concourse.tile as tile
from concourse import bacc, mybir
from concourse.bass_utils import run_bass_kernel_spmd
from concourse.masks import make_identity

B = 2048
LF = 64
FE = LF + 1  # weights + ones column
N_CORES = 8
BPC = B // N_CORES  # 256 batch rows per core
P = 128
SLAB = 32  # vocab chunks per A-slab DMA (32 -> 4 MiB int32 reads)

# (key, idx input name, weight input name, vocab size, output column offset)
# Processing order puts movies first so its row-sum reciprocal exists when
# decades is normalized.
GROUPS = [
    ("mov", "movie_idxs", "W_mov", 60000, 64),
    ("dec", "decade_idxs", "W_dec", 12, 0),
    ("cat", "category_idxs", "W_cat", 32, 128),
    ("per", "person_idxs", "W_per", 100000, 192),
    ("com", "company_idxs", "W_com", 20000, 256),
]
N_CH = [math.ceil(v / P) for _, _, _, v, _ in GROUPS]
C_TOT = sum(N_CH)  # 1410 chunks of 128 vocab rows
OUT_COLS = 5 * LF

_FP16 = mybir.dt.float16
_FP32 = mybir.dt.float32


def _build() -> bass.Bass:
    nc = bacc.Bacc(None, target_bir_lowering=False)

    a_dram = nc.dram_tensor("a_all", [P, C_TOT * 2 * P], mybir.dt.int32,
                            kind="ExternalInput")
    w_dram = nc.dram_tensor("w_all", [P, C_TOT * FE], _FP16,
                            kind="ExternalInput")
    out = nc.dram_tensor("out", [BPC, OUT_COLS], _FP32, kind="ExternalOutput")

    with tile.TileContext(nc) as tc:
        with (
            tc.tile_pool(name="singles", bufs=1) as singles,
            tc.tile_pool(name="apool", bufs=4) as apool,
            tc.tile_pool(name="wpool", bufs=4) as wpool,
            tc.tile_pool(name="npool", bufs=4) as npool,
            tc.tile_pool(name="accp", bufs=2, space="PSUM") as accp,
            tc.tile_pool(name="backp", bufs=2, space="PSUM") as backp,
        ):
            ident32 = singles.tile([P, P], _FP32)
            make_identity(nc, ident32)

            out_sb = [singles.tile([P, OUT_COLS], _FP32, name=f"out_sb{i}")
                      for i in range(2)]
            rmov = [singles.tile([P, 1], _FP32, name=f"rmov{i}")
                    for i in range(2)]

            cur_acc = {}  # group key -> live PSUM accumulator tile
            for c0 in range(0, C_TOT, SLAB):
                ch = min(SLAB, C_TOT - c0)
                a_sb = apool.tile([P, SLAB, 2 * P], _FP16, tag="a")
                nc.gpsimd.dma_start(
                    a_sb[:, :ch, :],
                    a_dram[:, c0 * 2 * P:(c0 + ch) * 2 * P].rearrange(
                        "p (c b) -> p c b", b=2 * P),
                )
                w_sb = wpool.tile([P, SLAB, FE], _FP16, tag="w")
                nc.sync.dma_start(
                    w_sb[:, :ch, :],
                    w_dram[:, c0 * FE:(c0 + ch) * FE].rearrange(
                        "p (c f) -> p c f", f=FE),
                )
                # all chunks of this slab: accumulate into the owning group's
                # PSUM accumulator
                for j in range(ch):
                    cidx = c0 + j
                    # map chunk -> group
                    g0 = 0
                    for gi, nch in enumerate(N_CH):
                        if cidx < g0 + nch:
                            break
                        g0 += nch
                    key, _, _, v, col = GROUPS[gi]
                    if cidx == g0:
                        cur_acc[key] = accp.tile([FE, 2 * P], _FP32, tag="acc",
                                                 name=f"accT_{key}")
                    accT = cur_acc[key]
                    nc.tensor.matmul(
                        accT,
                        lhsT=w_sb[:, j, :],
                        rhs=a_sb[:, j, :],
                        start=(cidx == g0),
                        stop=(cidx == g0 + N_CH[gi] - 1),
                    )
                    if cidx != g0 + N_CH[gi] - 1:
                        continue

                    # group epilogue: back-transpose, normalize, stage output
                    accT_sb = npool.tile([FE, 2 * P], _FP32, tag="accsb")
                    nc.vector.tensor_copy(accT_sb, accT)
                    for bt in range(2):
                        out2 = backp.tile([P, FE], _FP32, tag="out2")
                        nc.tensor.matmul(
                            out2,
                            lhsT=accT_sb[:, bass.ts(bt, P)],
                            rhs=ident32[:FE, :FE],
                            start=True, stop=True,
                        )
                        s = npool.tile([P, 1], _FP32, tag="s")
                        nc.vector.tensor_scalar_max(s, out2[:, LF:FE], 1.0)
                        nc.vector.reciprocal(s, s)
                        if key == "mov":
                            # movies are left unnormalized; stash 1/max(sum,1)
                            # for the decades double-normalization
                            nc.vector.tensor_copy(rmov[bt], s)
                            nc.scalar.copy(out_sb[bt][:, col:col + LF],
                                           out2[:, :LF])
                        else:
                            if key == "dec":
                                nc.vector.tensor_mul(s, s, rmov[bt])
                            nc.vector.tensor_scalar_mul(
                                out_sb[bt][:, col:col + LF], out2[:, :LF], s)

            for bt in range(2):
                nc.sync.dma_start(out[bt * P:(bt + 1) * P, :], out_sb[bt])

    nc.finalize()
    return nc


_NC_CACHE: bass.Bass | None = None


def _get_nc() -> bass.Bass:
    global _NC_CACHE
    if _NC_CACHE is None:
        _NC_CACHE = _build()
    return _NC_CACHE


def _pack_weights(inputs: dict) -> np.ndarray:
    """Concat groups' [W_g | 1] fp16 chunk-major into [128, C_TOT*65]."""
    w_all = np.zeros((P, C_TOT, FE), np.float16)
    c0 = 0
    for (_, _, wname, v, _), c in zip(GROUPS, N_CH):
        we = np.concatenate(
            [np.asarray(inputs[wname], np.float32),
             np.ones((v, 1), np.float32)], axis=1).astype(np.float16)
        if c * P > v:
            we = np.concatenate(
                [we, np.zeros((c * P - v, FE), np.float16)], axis=0)
        w_all[:, c0:c0 + c, :] = we.reshape(c, P, FE).transpose(1, 0, 2)
        c0 += c
    return np.ascontiguousarray(w_all.reshape(P, C_TOT * FE))


def _pack_a(inputs: dict) -> np.ndarray:
    """Host transpose (int32 preserved): per core, vocab chunks on partitions.

    Returns [N_CORES, 128, C_TOT, 2*P] int32 where [core, p, c0g+c, b] =
    A_g[core*256 + b, c*128 + p] (zero beyond each group's vocab)."""
    arr = np.zeros((N_CORES, P, C_TOT, 2 * P), np.int32)
    c0 = 0
    for (_, aname, _, v, _), c in zip(GROUPS, N_CH):
        a = np.asarray(inputs[aname], np.int32).reshape(N_CORES, 2 * P, v)
        fc = v // P
        if fc:
            arr[:, :, c0:c0 + fc, :] = (
                a[:, :, :fc * P].reshape(N_CORES, 2 * P, fc, P)
                .transpose(0, 3, 2, 1))
        if v % P:
            arr[:, :v % P, c0 + fc, :] = a[:, :, fc * P:].transpose(0, 2, 1)
        c0 += c
    return arr


def kernel(**inputs: np.ndarray) -> np.ndarray:
    import os

    nc = _get_nc()

    w_all = _pack_weights(inputs)
    a_all = _pack_a(inputs)

    in_maps = []
    for core in range(N_CORES):
        in_maps.append({
            "a_all": a_all[core].reshape(P, C_TOT * 2 * P),
            "w_all": w_all,
        })

    trace = bool(int(os.environ.get("EMB_TRACE", "0")))
    res = run_bass_kernel_spmd(nc, in_maps, core_ids=list(range(N_CORES)),
                               trace=trace)
    if trace and res.exec_time_ns is not None:
        print(f"HW exec time: {res.exec_time_ns} ns")
        if res.instructions_and_trace is not None:
            print(f"trace: {res.instructions_and_trace[1]}")

    return np.concatenate([r["out"] for r in res.results], axis=0)
